# revision 9
# baseline (speedup 1.0000x reference)
"""ChildSumTreeLSTM (perfect binary tree) Trainium2 kernel.

Problem: B=8 trees, 16384 leaves/tree, D_IN=768, D_H=128.
  leaves:  h = x @ W_in + b_in, c = 0
  level:   h_avg = mean of child pair; gates = h_avg @ W_up + b_up
           i,o,f = sigmoid; u = tanh; c' = i*u + f*(c1+c2); h' = o*tanh(c')
Returns (h_root, c_root), each [B, 128].

Sharding: data-parallel, one tree per NeuronCore (8 cores).

Per-core kernel layout: everything transposed — feature dim on SBUF
partitions, node index on the free axis.  Host pre-transposes x to
[din, leaves] (tiled for DMA) so the leaf projection is a plain
contraction-on-partition matmul chain with no on-device transposes.

Algebraic folds (all exact in fp32):
  - leaf c = 0 and leaf h is only consumed through pair means, so the
    leaf bias b_in folds into the level-0 gate bias:
        bias0 = b_in @ W_up + b_up
    (and the level-0 f gate multiplies c==0, so it is skipped)
  - pair MEAN folds into the gate weight: W1 = 0.5 * W_up, and the
    pair SUM is computed for free by two accumulating matmuls whose
    moving operands are the stride-2 even/odd views of the child h.

Matmuls run in float32r (TF32-class, full PE rate); the c state and
all element-wise math stay fp32.
"""

import sys

sys.path.insert(0, "/opt/trn_rl_repo")

import numpy as np

try:  # persistent executable cache: repeat runs skip the multi-minute NEFF compile
    import jax as _jax

    _jax.config.update("jax_compilation_cache_dir", "/tmp/jax_neff_cache")
    _jax.config.update("jax_persistent_cache_min_compile_time_secs", 10.0)
except Exception:
    pass

import concourse.bass as bass
import concourse.bacc as bacc
import concourse.mybir as mybir
from concourse import tile
from concourse.bass_utils import run_bass_kernel_spmd

AF = mybir.ActivationFunctionType
F32 = mybir.dt.float32

N_CORES = 8
D_IN = 768
D_H = 128
N_LEAVES = 16384
F_LEAF = 512  # leaves per DMA/compute chunk
F_TREE = 512  # max free-dim per tree-level chunk
KCH = D_IN // 128  # k-chunks of the leaf contraction

# i, o, u, f gate order (reference splits gates in this order)
GATE_FUNCS = (AF.Sigmoid, AF.Sigmoid, AF.Tanh, AF.Sigmoid)


def build_nc(n_leaves=N_LEAVES, mm_dt=mybir.dt.float32r, f_leaf=F_LEAF):
    nc = bacc.Bacc("TRN2", target_bir_lowering=False, debug=False)
    n_chunks = n_leaves // f_leaf
    p_chunk = f_leaf // 2  # level-0 parents per leaf chunk

    x_d = nc.dram_tensor("xt", [n_chunks, KCH, 128, f_leaf], mm_dt, kind="ExternalInput")
    win_d = nc.dram_tensor("w_in", [KCH, 128, D_H], mm_dt, kind="ExternalInput")
    w1_d = nc.dram_tensor("w1", [D_H, 4 * D_H], mm_dt, kind="ExternalInput")
    bias_d = nc.dram_tensor("bias", [8, 128], F32, kind="ExternalInput")
    out_d = nc.dram_tensor("out", [2, D_H], F32, kind="ExternalOutput")

    with tile.TileContext(nc) as tc:
        with (
            tc.tile_pool(name="const", bufs=1) as cpool,
            tc.tile_pool(name="state", bufs=1) as bpool,
            tc.tile_pool(name="work", bufs=2) as wpool,
            tc.tile_pool(name="hs_ps", bufs=2, space=bass.MemorySpace.PSUM) as ppool,
            tc.tile_pool(name="g_ps", bufs=1, space=bass.MemorySpace.PSUM) as gpool,
        ):
            w_in = cpool.tile([128, KCH, D_H], mm_dt, tag="w_in")
            nc.sync.dma_start(out=w_in[:], in_=win_d.rearrange("k p m -> p k m"))
            w1 = cpool.tile([128, 4 * D_H], mm_dt, tag="w1")
            nc.sync.dma_start(out=w1[:], in_=w1_d[:])
            bias = cpool.tile([128, 8], F32, tag="bias")
            nc.sync.dma_start(out=bias[:], in_=bias_d.rearrange("i p -> p i"))

            def node_update(F, h_out, c_out, hs=None, rhs_pair=None, cs=None, lvl0=False):
                """One batch of F parent nodes: gates -> (h_out, c_out)."""
                bb = 0 if lvl0 else 4
                # fp32r matmul requires an even innermost element count; the
                # odd-F tail (root level, F==1) falls back to plain fp32.
                cast = (lambda ap: ap.bitcast(F32)) if F % 2 else (lambda ap: ap)
                acts = []
                for g in range(3 if lvl0 else 4):  # lvl0: f gate multiplies c==0
                    ps = gpool.tile([128, F], F32, tag=f"g{g}")
                    w = cast(w1[:, g * D_H : (g + 1) * D_H])
                    if hs is not None:
                        nc.tensor.matmul(ps[:], w, cast(hs), start=True, stop=True)
                    else:
                        nc.tensor.matmul(ps[:], w, cast(rhs_pair[0]), start=True, stop=False)
                        nc.tensor.matmul(ps[:], w, cast(rhs_pair[1]), start=False, stop=True)
                    a = wpool.tile([128, F], F32, tag=f"a{g}")
                    nc.scalar.activation(
                        a[:], ps[:], GATE_FUNCS[g], bias=bias[:, bb + g : bb + g + 1]
                    )
                    acts.append(a)
                if cs is None:  # children carry c == 0
                    i_t, o_t, u_t = acts
                    nc.vector.tensor_mul(c_out, i_t[:], u_t[:])
                else:
                    i_t, o_t, u_t, f_t = acts
                    iu = wpool.tile([128, F], F32, tag="iu")
                    nc.vector.tensor_mul(iu[:], i_t[:], u_t[:])
                    fcs = wpool.tile([128, F], F32, tag="fcs")
                    nc.vector.tensor_mul(fcs[:], f_t[:], cs[:])
                    nc.vector.tensor_add(c_out, iu[:], fcs[:])
                t = wpool.tile([128, F], F32, tag="t")
                nc.scalar.activation(t[:], c_out, AF.Tanh)
                nc.vector.tensor_mul(h_out, o_t[:], t[:])

            # ---- leaf projection fused with level 0 ----
            n0 = n_leaves // 2
            h_cur = bpool.tile([128, n0], mm_dt, tag="h_even")
            c_cur = bpool.tile([128, n0], F32, tag="c_even")
            for ci in range(n_chunks):
                xt = wpool.tile([128, KCH, f_leaf], mm_dt, tag="xt")
                nc.sync.dma_start(out=xt[:], in_=x_d[ci].rearrange("k p n -> p k n"))
                xv = xt.rearrange("p k (n two) -> p k n two", two=2)
                hs_ps = ppool.tile([128, p_chunk], F32, tag="hs_ps")
                for k in range(KCH):
                    for two in range(2):
                        nc.tensor.matmul(
                            hs_ps[:],
                            w_in[:, k, :],
                            xv[:, k, :, two],
                            start=(k == 0 and two == 0),
                            stop=(k == KCH - 1 and two == 1),
                        )
                hs = wpool.tile([128, p_chunk], mm_dt, tag="hs")
                nc.scalar.copy(hs[:], hs_ps[:])
                sl = slice(ci * p_chunk, (ci + 1) * p_chunk)
                node_update(p_chunk, h_cur[:, sl], c_cur[:, sl], hs=hs[:], lvl0=True)

            # ---- tree levels 1.. ----
            n = n0
            parity = 0
            while n > 1:
                n_next = n // 2
                parity ^= 1
                pname = "odd" if parity else "even"
                h_next = bpool.tile([128, n_next], mm_dt, tag=f"h_{pname}")
                c_next = bpool.tile([128, n_next], F32, tag=f"c_{pname}")
                hv = h_cur.rearrange("p (n two) -> p n two", two=2)
                cv = c_cur.rearrange("p (n two) -> p n two", two=2)
                for c0 in range(0, n_next, F_TREE):
                    F = min(F_TREE, n_next - c0)
                    cs = wpool.tile([128, F], F32, tag="cs")
                    nc.vector.tensor_add(
                        cs[:], cv[:, c0 : c0 + F, 0], cv[:, c0 : c0 + F, 1]
                    )
                    node_update(
                        F,
                        h_next[:, c0 : c0 + F],
                        c_next[:, c0 : c0 + F],
                        rhs_pair=(hv[:, c0 : c0 + F, 0], hv[:, c0 : c0 + F, 1]),
                        cs=cs[:],
                    )
                h_cur, c_cur, n = h_next, c_next, n_next

            nc.sync.dma_start(
                out=out_d[0:1, :].rearrange("a p -> p a"), in_=h_cur[:].bitcast(F32)
            )
            nc.sync.dma_start(out=out_d[1:2, :].rearrange("a p -> p a"), in_=c_cur[:])

    nc.compile()
    return nc


def prep_inputs(x, W_in, b_in, W_up, b_up, n_leaves=N_LEAVES, f_leaf=F_LEAF):
    """Host-side fold + per-core shard maps."""
    x = np.asarray(x, dtype=np.float32)
    W_in = np.asarray(W_in, dtype=np.float32)
    b_in = np.asarray(b_in, dtype=np.float32)
    W_up = np.asarray(W_up, dtype=np.float32)
    b_up = np.asarray(b_up, dtype=np.float32)

    n_chunks = n_leaves // f_leaf
    w1 = np.ascontiguousarray(0.5 * W_up)
    bias0 = b_in @ W_up + b_up
    bias_h = np.ascontiguousarray(
        np.concatenate([bias0, b_up]).reshape(8, D_H).astype(np.float32)
    )
    win_h = np.ascontiguousarray(W_in.reshape(KCH, 128, D_H))

    in_maps = []
    for i in range(x.shape[0]):
        # [n, din] -> [din, n] -> [KCH, 128, n_chunks, f_leaf] -> [n_chunks, KCH, 128, f_leaf]
        xt = np.ascontiguousarray(
            x[i].T.reshape(KCH, 128, n_chunks, f_leaf).transpose(2, 0, 1, 3)
        )
        in_maps.append({"xt": xt, "w_in": win_h, "w1": w1, "bias": bias_h})
    return in_maps


_NC_CACHE = {}


def kernel(x, W_in, b_in, W_up, b_up):
    x = np.asarray(x, dtype=np.float32)
    B = x.shape[0]
    assert B == N_CORES and x.shape[1] == N_LEAVES and x.shape[2] == D_IN

    key = (N_LEAVES,)
    if key not in _NC_CACHE:
        _NC_CACHE[key] = build_nc(N_LEAVES)
    nc = _NC_CACHE[key]

    in_maps = prep_inputs(x, W_in, b_in, W_up, b_up)
    res = run_bass_kernel_spmd(nc, in_maps, list(range(N_CORES)))
    out = np.stack([res.results[i]["out"] for i in range(N_CORES)])  # [B, 2, 128]
    return out[:, 0].astype(np.float32), out[:, 1].astype(np.float32)


# revision 32
# speedup vs baseline: 20.6903x; 20.6903x over previous
"""ChildSumTreeLSTM (perfect binary tree) Trainium2 kernel.

Problem: B=8 trees, 16384 leaves/tree, D_IN=768, D_H=128.
  leaves:  h = x @ W_in + b_in, c = 0
  level:   h_avg = mean of child pair; gates = h_avg @ W_up + b_up
           i,o,f = sigmoid; u = tanh; c' = i*u + f*(c1+c2); h' = o*tanh(c')
Returns (h_root, c_root), each [B, 128].

Sharding: data-parallel, one tree per NeuronCore (8 cores).

Per-core kernel layout: everything transposed — feature dim on SBUF
partitions, node index on the free axis.  Host pre-transposes x to
[din, leaves] (tiled for DMA) so the leaf projection is a plain
contraction-on-partition matmul chain with no on-device transposes.

Algebraic folds (all exact in fp32):
  - leaf c = 0 and leaf h is only consumed through pair means, so the
    leaf bias b_in folds into the level-0 gate bias:
        bias0 = b_in @ W_up + b_up
    (and the level-0 f gate multiplies c==0, so it is skipped)
  - pair MEAN folds into the gate weight: W1 = 0.5 * W_up, and the
    pair SUM is computed for free by two accumulating matmuls whose
    moving operands are the stride-2 even/odd views of the child h.
  - gate biases ride the same PSUM accumulation group as a rank-1
    matmul (bias ⊗ ones), so the i/o/f sigmoids collapse into one
    bias-free activation op over a merged PSUM tile.

The tree is emitted as a pipelined cascade: a level-l chunk is emitted
as soon as its level-(l-1) input range exists, so upper levels overlap
the leaf DMA stream and only the right spine trails the last chunk.

Matmuls run in float32r (TF32-class, full PE rate); the c state and
all element-wise math stay fp32.  Gates are ordered [i, o, f, u].
"""

import sys

sys.path.insert(0, "/opt/trn_rl_repo")

import numpy as np

try:  # persistent executable cache: repeat runs skip the multi-minute NEFF compile
    import jax as _jax

    _jax.config.update("jax_compilation_cache_dir", "/tmp/jax_neff_cache")
    _jax.config.update("jax_persistent_cache_min_compile_time_secs", 10.0)
except Exception:
    pass

import concourse.bass as bass
import concourse.bacc as bacc
import concourse.mybir as mybir
from concourse import tile
from concourse.bass_utils import run_bass_kernel_spmd

AF = mybir.ActivationFunctionType
F32 = mybir.dt.float32

N_CORES = 8
D_IN = 768
D_H = 128
N_LEAVES = 16384
F_LEAF = 512  # leaves per DMA/compute chunk
F_TREE = 256  # free-dim per tree-level chunk
KCH = D_IN // 128  # k-chunks of the leaf contraction


def build_nc(n_leaves=N_LEAVES, mm_dt=mybir.dt.float32r, f_leaf=F_LEAF,
             f_tree=F_TREE, merge_gates=True, taper="none", bias_mm_min_f=0,
             xt_bufs=3, x_dt=None, pool_pair=False, lvl0_f=256, reps=1):
    """x_dt: dtype of the x / W_in leaf-projection path (default mm_dt;
    bfloat16 halves the DMA floor at ~3e-3 leaf precision)."""
    x_dt = x_dt or mm_dt
    nc = bacc.Bacc("TRN2", target_bir_lowering=False, debug=False)
    n_chunks = n_leaves // f_leaf
    p_chunk = f_leaf // 2  # level-0 parents per leaf chunk

    x_d = nc.dram_tensor("xt", [n_chunks, KCH, 128, f_leaf], x_dt, kind="ExternalInput")
    win_d = nc.dram_tensor("w_in", [KCH, 128, D_H], x_dt, kind="ExternalInput")
    w1_d = nc.dram_tensor("w1", [D_H, 4 * D_H], mm_dt, kind="ExternalInput")
    bias_d = nc.dram_tensor("bias", [8, 128], mm_dt, kind="ExternalInput")
    ones_d = nc.dram_tensor("ones", [512], mm_dt, kind="ExternalInput")
    out_d = nc.dram_tensor("out", [2, D_H], F32, kind="ExternalOutput")

    # level sizes: ns[l] parents at level l (level 0 consumes leaf pairs)
    ns = []
    n = n_leaves // 2
    while n >= 1:
        ns.append(n)
        if n == 1:
            break
        n //= 2
    n_levels = len(ns)

    with tile.TileContext(nc) as tc:
        with (
            tc.tile_pool(name="const", bufs=1) as cpool,
            tc.tile_pool(name="state", bufs=1) as bpool,
            tc.tile_pool(name="work", bufs=2) as wpool,
            tc.tile_pool(name="hs_ps", bufs=2, space=bass.MemorySpace.PSUM) as ppool,
            tc.tile_pool(name="g_ps", bufs=2, space=bass.MemorySpace.PSUM) as gpool,
        ):
            w_in = cpool.tile([128, KCH, D_H], x_dt, tag="w_in")
            nc.sync.dma_start(out=w_in[:], in_=win_d.rearrange("k p m -> p k m"))
            w1 = cpool.tile([128, 4 * D_H], mm_dt, tag="w1")
            nc.sync.dma_start(out=w1[:], in_=w1_d[:])
            # full-height allocations (row 0 used): a <128-partition tile can
            # land at base_partition>0, which matmul lhsT auto-tiling rejects
            bias_row_t = cpool.tile([128, 8 * D_H], mm_dt, tag="bias_row")
            bias_row = bias_row_t[0:1, :]
            nc.sync.dma_start(out=bias_row, in_=bias_d.rearrange("i p -> (i p)"))
            ones_t = cpool.tile([128, 512], mm_dt, tag="ones")
            ones = ones_t[0:1, :]
            nc.sync.dma_start(out=ones, in_=ones_d.rearrange("(a n) -> a n", a=1))
            bias_col = cpool.tile([128, 8], F32, tag="bias_col")
            nc.sync.dma_start(out=bias_col[:], in_=bias_d.rearrange("i p -> p i").bitcast(F32))

            # per-level state buffers (distinct allocations so upper levels can
            # run pipelined against lower ones without slot WAR serialization)
            h_buf = [
                bpool.tile([128, ns[l]], mm_dt, tag=f"h{l}", name=f"h{l}")
                for l in range(n_levels)
            ]
            c_buf = [
                bpool.tile([128, ns[l]], F32, tag=f"c{l}", name=f"c{l}")
                for l in range(n_levels)
            ]

            def node_update(F, h_out, c_out, hs=None, rhs_pair=None, cs=None, lvl0=False):
                """One batch of F parent nodes: gates -> (h_out, c_out)."""
                nsig = 2 if lvl0 else 3  # merged sigmoid gates: i,o(,f)
                bb = 0 if lvl0 else 4  # bias row base
                # fp32r matmul requires an even innermost element count; the
                # odd-F tail (root level, F==1) falls back to plain fp32.
                cast = (lambda ap: ap.bitcast(F32)) if F % 2 else (lambda ap: ap)

                use_bias_mm = merge_gates and F > bias_mm_min_f

                def gate_group(dst, g, with_bias_mm):
                    w = cast(w1[:, g * D_H : (g + 1) * D_H])
                    if with_bias_mm:
                        b = bias_row[:, (bb + g) * D_H : (bb + g + 1) * D_H]
                        nc.tensor.matmul(dst, cast(b), cast(ones[:, 0:F]),
                                         start=True, stop=False)
                    if hs is not None:
                        nc.tensor.matmul(dst, w, cast(hs),
                                         start=not with_bias_mm, stop=True)
                    else:
                        nc.tensor.matmul(dst, w, cast(rhs_pair[0]),
                                         start=not with_bias_mm, stop=False)
                        nc.tensor.matmul(dst, w, cast(rhs_pair[1]), start=False, stop=True)

                gb = 1 if max(f_tree, lvl0_f) > 256 else 2
                ps = gpool.tile([128, 3 * F], F32, tag="giof", bufs=gb)
                psu = gpool.tile([128, F], F32, tag="gu", bufs=gb)
                a_sig = wpool.tile([128, nsig * F], F32, tag="asig")
                u_t = wpool.tile([128, F], F32, tag="ut")
                if use_bias_mm:
                    for g in range(nsig):
                        gate_group(ps[:, g * F : (g + 1) * F], g, True)
                    gate_group(psu[:], 3, True)
                    nc.scalar.activation(a_sig[:], ps[:, 0 : nsig * F], AF.Sigmoid)
                    nc.scalar.activation(u_t[:], psu[:], AF.Tanh)
                else:
                    for g in range(nsig):
                        gate_group(ps[:, g * F : (g + 1) * F], g, False)
                        nc.scalar.activation(
                            a_sig[:, g * F : (g + 1) * F],
                            ps[:, g * F : (g + 1) * F],
                            AF.Sigmoid,
                            bias=bias_col[:, bb + g : bb + g + 1],
                        )
                    gate_group(psu[:], 3, False)
                    nc.scalar.activation(u_t[:], psu[:], AF.Tanh,
                                         bias=bias_col[:, bb + 3 : bb + 4])
                i_t = a_sig[:, 0:F]
                o_t = a_sig[:, F : 2 * F]
                if cs is None:  # children carry c == 0
                    nc.vector.tensor_mul(c_out, i_t, u_t[:])
                else:
                    f_t = a_sig[:, 2 * F : 3 * F]
                    iu = wpool.tile([128, F], F32, tag="iu")
                    nc.vector.tensor_mul(iu[:], i_t, u_t[:])
                    fcs = wpool.tile([128, F], F32, tag="fcs")
                    nc.gpsimd.tensor_mul(fcs[:], f_t, cs)
                    nc.vector.tensor_add(c_out, iu[:], fcs[:])
                t = wpool.tile([128, F], F32, tag="t")
                nc.scalar.activation(t[:], c_out, AF.Tanh)
                nc.vector.tensor_mul(h_out, o_t, t[:])

            def emit_tree_chunk(l, j0, F):
                """Level-l parents [j0, j0+F) from level l-1 children."""
                hv = h_buf[l - 1].rearrange("p (n two) -> p n two", two=2)
                cv = c_buf[l - 1].rearrange("p (n two) -> p n two", two=2)
                cs = wpool.tile([128, F], F32, tag="cs")
                nc.gpsimd.tensor_add(cs[:], cv[:, j0 : j0 + F, 0], cv[:, j0 : j0 + F, 1])
                if pool_pair and F % 2 == 0:
                    hsum = wpool.tile([128, F], mm_dt, tag="hsum")
                    nc.gpsimd.tensor_add(
                        hsum[:], hv[:, j0 : j0 + F, 0], hv[:, j0 : j0 + F, 1]
                    )
                    node_update(
                        F,
                        h_buf[l][:, j0 : j0 + F],
                        c_buf[l][:, j0 : j0 + F],
                        hs=hsum[:],
                        cs=cs[:],
                    )
                else:
                    node_update(
                        F,
                        h_buf[l][:, j0 : j0 + F],
                        c_buf[l][:, j0 : j0 + F],
                        rhs_pair=(hv[:, j0 : j0 + F, 0], hv[:, j0 : j0 + F, 1]),
                        cs=cs[:],
                    )

            emitted = [0] * n_levels  # parents emitted per level

            def level_pieces(l, n):
                """Bulk f_tree chunks, plus (policy-dependent) narrow final
                pieces: the last-emitted pieces form the kernel's tail chain,
                so their width sets the tail latency."""
                if taper == "cone":
                    # final piece = ancestor cone of the last leaf chunk
                    cone = max(256 >> l, 1)
                    if n <= cone:
                        return [n]
                    out = []
                    rem = n - cone
                    while rem > f_tree:
                        out.append(f_tree)
                        rem -= f_tree
                    if rem:
                        out.append(rem)
                    out.append(cone)
                    return out
                do_taper = taper == "all" or (taper == "small" and n <= 512)
                out = []
                rem = n
                while rem > f_tree:
                    out.append(f_tree)
                    rem -= f_tree
                if do_taper:
                    while rem > 32:
                        out.append(rem // 2)
                        rem -= rem // 2
                if rem:
                    out.append(rem)
                return out

            piece_plan = [None] + [level_pieces(l, ns[l]) for l in range(1, n_levels)]
            piece_idx = [0] * n_levels

            def cascade():
                """Emit every upper-level piece whose inputs are complete."""
                for l in range(1, n_levels):
                    plan = piece_plan[l]
                    while piece_idx[l] < len(plan):
                        Fl = plan[piece_idx[l]]
                        if 2 * (emitted[l] + Fl) > emitted[l - 1]:
                            break
                        emit_tree_chunk(l, emitted[l], Fl)
                        emitted[l] += Fl
                        piece_idx[l] += 1

            # ---- leaf projection fused with level 0, cascading upward ----
            # All consumer work is emitted with a one-leaf-chunk lag so that
            # by the time an instruction enters its engine FIFO, its inputs
            # are already computed — otherwise a waiting tree matmul
            # head-of-line-blocks the next leaf chunk's matmuls in the
            # in-order PE queue and the DMA stream stalls.
            kh = KCH // 2

            def collect_ready():
                """Pop every tree piece whose inputs were emitted in PRIOR
                batches (snapshot) — a piece depending on a same-batch piece
                would head-of-line-block the engine FIFOs."""
                snap = list(emitted)
                out = []
                for l in range(1, n_levels):
                    plan = piece_plan[l]
                    while piece_idx[l] < len(plan):
                        Fl = plan[piece_idx[l]]
                        if 2 * (emitted[l] + Fl) > snap[l - 1]:
                            break
                        out.append((l, emitted[l], Fl))
                        emitted[l] += Fl
                        piece_idx[l] += 1
                return out

            def _emit_main():
              hs_ring = {}
              ready = []
              emitted[:] = [0] * n_levels
              piece_idx[:] = [0] * n_levels
              for ci in range(n_chunks + 1):
                if ci < n_chunks:
                    xt = wpool.tile([128, KCH, f_leaf], x_dt, tag="xt", bufs=xt_bufs)
                    # two k-half DMAs so matmuls can start on the first half
                    nc.sync.dma_start(
                        out=xt[:, 0:kh, :], in_=x_d[ci][0:kh].rearrange("k p n -> p k n")
                    )
                    nc.sync.dma_start(
                        out=xt[:, kh:KCH, :],
                        in_=x_d[ci][kh:KCH].rearrange("k p n -> p k n"),
                    )
                    xv = xt.rearrange("p k (n two) -> p k n two", two=2)
                    hs_ps = ppool.tile([128, p_chunk], F32, tag="hs_ps")
                    for k in range(KCH):
                        for two in range(2):
                            nc.tensor.matmul(
                                hs_ps[:],
                                w_in[:, k, :],
                                xv[:, k, :, two],
                                start=(k == 0 and two == 0),
                                stop=(k == KCH - 1 and two == 1),
                            )
                    hs = wpool.tile([128, p_chunk], mm_dt, tag="hs", bufs=3)
                    nc.vector.tensor_copy(hs[:], hs_ps[:])
                    hs_ring[ci] = hs
                if ci >= 1:  # lagged level-0 update for the previous chunk
                    cj = ci - 1
                    hs_t = hs_ring.pop(cj)
                    f0 = min(lvl0_f, p_chunk)
                    for s in range(p_chunk // f0):
                        j0 = cj * p_chunk + s * f0
                        node_update(
                            f0, h_buf[0][:, j0 : j0 + f0], c_buf[0][:, j0 : j0 + f0],
                            hs=hs_t[:, s * f0 : (s + 1) * f0], lvl0=True,
                        )
                    emitted[0] += p_chunk
                for l, j0, Fl in ready:  # lagged cascade pieces
                    emit_tree_chunk(l, j0, Fl)
                ready = collect_ready()
              while ready:
                for l, j0, Fl in ready:
                    emit_tree_chunk(l, j0, Fl)
                ready = collect_ready()

              assert all(emitted[l] == ns[l] for l in range(n_levels)), emitted

              nc.sync.dma_start(
                  out=out_d[0:1, :].rearrange("a p -> p a"), in_=h_buf[-1][:].bitcast(F32)
              )
              nc.sync.dma_start(out=out_d[1:2, :].rearrange("a p -> p a"), in_=c_buf[-1][:])

            if reps == 1:
                _emit_main()
            else:  # timing-calibration builds: repeat the whole body
                with tc.For_i(0, reps, 1):
                    _emit_main()

    nc.compile()
    return nc


# W_up/bias gate permutation [i, o, u, f] -> [i, o, f, u]
_GPERM = (0, 1, 3, 2)


def prep_inputs(x, W_in, b_in, W_up, b_up, n_leaves=N_LEAVES, f_leaf=F_LEAF,
                x_np_dtype=np.float32):
    """Host-side fold + per-core shard maps."""
    x = np.asarray(x, dtype=np.float32)
    W_in = np.asarray(W_in, dtype=np.float32)
    b_in = np.asarray(b_in, dtype=np.float32)
    W_up = np.asarray(W_up, dtype=np.float32)
    b_up = np.asarray(b_up, dtype=np.float32)

    n_chunks = n_leaves // f_leaf
    w1g = (0.5 * W_up).reshape(D_H, 4, D_H)[:, _GPERM, :]
    w1 = np.ascontiguousarray(w1g.reshape(D_H, 4 * D_H))
    bias0 = (b_in @ W_up + b_up).reshape(4, D_H)[_GPERM, :]
    biasr = b_up.reshape(4, D_H)[_GPERM, :]
    bias_h = np.ascontiguousarray(
        np.concatenate([bias0, biasr]).astype(np.float32)
    )
    win_h = np.ascontiguousarray(W_in.reshape(KCH, 128, D_H).astype(x_np_dtype))

    in_maps = []
    for i in range(x.shape[0]):
        # [n, din] -> [din, n] -> [KCH, 128, n_chunks, f_leaf] -> [n_chunks, KCH, 128, f_leaf]
        xt = np.ascontiguousarray(
            x[i].T.reshape(KCH, 128, n_chunks, f_leaf).transpose(2, 0, 1, 3)
        ).astype(x_np_dtype)
        in_maps.append({"xt": xt, "w_in": win_h, "w1": w1, "bias": bias_h,
                        "ones": np.ones(512, np.float32)})
    return in_maps


_NC_CACHE = {}

# chosen deployment config (x path dtype is decided by measured rel-err on HW)
USE_BF16_X = True


def _config(use_bf16=None):
    use_bf16 = USE_BF16_X if use_bf16 is None else use_bf16
    if use_bf16:
        import ml_dtypes

        return (
            dict(x_dt=mybir.dt.bfloat16, f_leaf=1024, f_tree=256, xt_bufs=3),
            dict(f_leaf=1024, x_np_dtype=ml_dtypes.bfloat16),
        )
    return dict(f_leaf=512, f_tree=256, xt_bufs=3), dict(f_leaf=512)


def kernel(x, W_in, b_in, W_up, b_up):
    x = np.asarray(x, dtype=np.float32)
    B = x.shape[0]
    assert B == N_CORES and x.shape[1] == N_LEAVES and x.shape[2] == D_IN

    build_kw, prep_kw = _config()
    key = (N_LEAVES, USE_BF16_X)
    if key not in _NC_CACHE:
        _NC_CACHE[key] = build_nc(N_LEAVES, **build_kw)
    nc = _NC_CACHE[key]

    in_maps = prep_inputs(x, W_in, b_in, W_up, b_up, **prep_kw)
    res = run_bass_kernel_spmd(nc, in_maps, list(range(N_CORES)))
    out = np.stack([res.results[i]["out"] for i in range(N_CORES)])  # [B, 2, 128]
    return out[:, 0].astype(np.float32), out[:, 1].astype(np.float32)
